# revision 6
# baseline (speedup 1.0000x reference)
"""MoE ExpertGroup kernel for Trainium2 (8 NeuronCores, expert-parallel).

Problem: E=8 experts, H=1024, I=4096, N=16384 tokens sorted by expert.
y[t] = gelu_tanh(x[t] @ w1[e(t)]) @ w2[e(t)]

Sharding: expert-parallel — core e holds expert e's weights and processes
expert e's contiguous token block (balanced routing: 2048 tokens/core).
The host ships each core's token block transposed (xT layout) and receives
the result transposed (yT) — transposition is part of the shard/unshard
step, so the device kernel is pure matmul+gelu.

Per-core dataflow (all matmuls in float32r — full-rate fp32 streaming):
  hT = gelu(w1.T @ xT)     MM1: lhsT=w1 tile [128,128], rhs=xT [128,512]
  yT = w2.T @ hT           MM2: lhsT=w2 tile, rhs=hT; PSUM-accumulated over
                            4-I-tile groups, DVE-accumulated across groups
Tokens are processed in 2 halves of 1024 to bound SBUF residency; the
second half's xT tiles are DMA'd during the first half's compute.
"""

import sys

sys.path.insert(0, "/opt/trn_rl_repo")

import numpy as np

# --- problem constants (hardcoded; kernel.py must be self-contained) ---
E = 8          # experts == cores
H = 1024       # hidden
I = 4096       # intermediate
N_TOK = 16384  # total tokens
T = N_TOK // E  # tokens per core (capacity)

P = 128
NH = 2               # token halves per core
TH = T // NH         # tokens per half (1024)
TB = 512             # token block (psum free dim)
NTB = TH // TB       # 2
HB = H // P          # 8
IB = I // P          # 32
GI = 4               # I-tiles per PSUM-accumulation group
NG = IB // GI        # 8 groups

_CACHE = {}


def _build():
    import concourse.bacc as bacc
    import concourse.mybir as mybir
    import concourse.tile as tile

    F32 = mybir.dt.float32
    F32R = mybir.dt.float32r
    GELU = mybir.ActivationFunctionType.Gelu_apprx_tanh

    nc = bacc.Bacc("TRN2", target_bir_lowering=False, debug=False, num_devices=E)

    xT = nc.dram_tensor("xT", [H, T], F32R, kind="ExternalInput").ap()
    w1 = nc.dram_tensor("w1", [H, I], F32R, kind="ExternalInput").ap()
    w2 = nc.dram_tensor("w2", [I, H], F32R, kind="ExternalInput").ap()
    yT = nc.dram_tensor("yT", [H, T], F32, kind="ExternalOutput").ap()

    with tile.TileContext(nc) as tc:
        with (
            tc.tile_pool(name="xTp", bufs=1) as xT_pool,
            tc.tile_pool(name="ysb", bufs=1) as y_pool,
            tc.tile_pool(name="w1p", bufs=2) as w1_pool,
            tc.tile_pool(name="w2p", bufs=2) as w2_pool,
            tc.tile_pool(name="hT", bufs=8) as hT_pool,
            tc.tile_pool(name="ph", bufs=4, space="PSUM") as ph_pool,
            tc.tile_pool(name="py", bufs=4, space="PSUM") as py_pool,
        ):
            # PE warmup: ~4us of dummy matmuls while the first DMAs land,
            # so the HAM clock gate is released before real work starts.
            wsrc = xT_pool.tile([P, TB], F32, tag="warm", name="wsrc")
            nc.gpsimd.memset(wsrc[:], 0.0)
            for wi in range(8):
                pw = ph_pool.tile([P, TB], F32, tag="ph", name="pw")
                nc.tensor.matmul(pw[:], wsrc[:, :P], wsrc[:], start=True, stop=True)

            for half in range(NH):
                t0 = half * TH

                # xT tiles for this half (own tags so half-2 loads overlap
                # half-1 compute)
                xTt = []
                for k in range(HB):
                    xt = xT_pool.tile([P, TH], F32R, tag=f"xT{half}_{k}", name=f"xT{half}_{k}")
                    for tb in range(NTB):
                        nc.sync.dma_start(
                            out=xt[:, tb * TB : (tb + 1) * TB],
                            in_=xT[k * P : (k + 1) * P, t0 + tb * TB : t0 + (tb + 1) * TB],
                        )
                    xTt.append(xt)

                ysb = [y_pool.tile([P, TH], F32, tag=f"y{h}", name=f"y{h}") for h in range(HB)]

                for g in range(NG):
                    w1t = []
                    for k in range(HB):
                        wt = w1_pool.tile([P, GI * P], F32R, tag=f"w1_{k}", name=f"w1_{k}")
                        nc.sync.dma_start(
                            out=wt[:],
                            in_=w1[k * P : (k + 1) * P, g * GI * P : (g + 1) * GI * P],
                        )
                        w1t.append(wt)
                    w2t = []
                    for il in range(GI):
                        i = g * GI + il
                        wt = w2_pool.tile([P, H], F32R, tag=f"w2_{il}", name=f"w2_{il}")
                        nc.sync.dma_start(out=wt[:], in_=w2[i * P : (i + 1) * P, :])
                        w2t.append(wt)

                    for tb in range(NTB):
                        ts_ = slice(tb * TB, (tb + 1) * TB)
                        hTt = []
                        for il in range(GI):
                            ph = ph_pool.tile([P, TB], F32, tag="ph", name="ph")
                            for k in range(HB):
                                nc.tensor.matmul(
                                    ph[:],
                                    w1t[k][:, il * P : (il + 1) * P],
                                    xTt[k][:, ts_],
                                    start=(k == 0),
                                    stop=(k == HB - 1),
                                )
                            ht = hT_pool.tile([P, TB], F32R, tag="ht", name="ht")
                            nc.scalar.activation(ht[:], ph[:], GELU)
                            hTt.append(ht)
                        for h in range(HB):
                            py = py_pool.tile([P, TB], F32, tag="py", name="py")
                            for il in range(GI):
                                nc.tensor.matmul(
                                    py[:],
                                    w2t[il][:, h * P : (h + 1) * P],
                                    hTt[il][:],
                                    start=(il == 0),
                                    stop=(il == GI - 1),
                                )
                            if g == 0:
                                nc.scalar.activation(
                                    ysb[h][:, ts_], py[:], mybir.ActivationFunctionType.Copy
                                )
                            else:
                                nc.vector.tensor_add(ysb[h][:, ts_], ysb[h][:, ts_], py[:])

                for h in range(HB):
                    for tb in range(NTB):
                        nc.sync.dma_start(
                            out=yT[h * P : (h + 1) * P, t0 + tb * TB : t0 + (tb + 1) * TB],
                            in_=ysb[h][:, tb * TB : (tb + 1) * TB],
                        )

    nc.compile()
    return nc


def _get_nc():
    if "nc" not in _CACHE:
        _CACHE["nc"] = _build()
    return _CACHE["nc"]


def kernel(x_sorted, w1, w2, expert_counts, local_expert_indices, **_unused):
    from concourse.bass_utils import run_bass_kernel_spmd

    x_sorted = np.ascontiguousarray(x_sorted, dtype=np.float32)
    w1 = np.ascontiguousarray(w1, dtype=np.float32)
    w2 = np.ascontiguousarray(w2, dtype=np.float32)
    counts = np.asarray(expert_counts, dtype=np.int64)

    n = x_sorted.shape[0]
    offsets = np.cumsum(counts)
    # per-token expert id, identical to reference's searchsorted
    eid = np.searchsorted(offsets, np.arange(n), side="right")

    nc = _get_nc()

    in_maps = []
    row_idx = []
    for e in range(E):
        rows = np.nonzero(eid == e)[0]
        assert len(rows) <= T, f"expert {e} overflows capacity {T}"
        xe = np.zeros((T, H), dtype=np.float32)
        xe[: len(rows)] = x_sorted[rows]
        row_idx.append(rows)
        in_maps.append(
            {"xT": np.ascontiguousarray(xe.T), "w1": w1[e], "w2": w2[e]}
        )

    res = run_bass_kernel_spmd(nc, in_maps, list(range(E))).results

    out = np.zeros((n, H), dtype=np.float32)
    for e in range(E):
        rows = row_idx[e]
        ye = np.ascontiguousarray(res[e]["yT"].T)
        out[rows] = ye[: len(rows)]
    return out
